# revision 15
# baseline (speedup 1.0000x reference)
"""ContextualLoss forward on 8 trn2 NeuronCores.

Problem: X, Y [4, 256, 64, 64] f32 ->  loss [4] f32
  y_mean[c] = mean_hw(Y);  Xc = X - y_mean; Yc = Y - y_mean
  Xn, Yn: L2-normalized over C per spatial position; S = Xn^T @ Yn  [N, N], N=4096
  d = 1 - S; dmin = row min d; w = exp((1 - d/(dmin+1e-3))/0.1); A = w/rowsum(w)
  loss_b = -log(mean_n max_m A[n, m])

Key algebra (per row n), with s = Xc^T @ Yn (X centered but unnormalized,
g = 1/||Xc||, smax = row max s):
  max_m A[n,:] = 1 / sum_m exp(a_n*(s_nm - smax_n)),
  a_n = 10*g_n/(1.001 - smax_n*g_n) = rr*(-10g),  rr = 1/(smax*g - 1.001)
  bias  = -a*smax = 10 + 10.01*rr   (exact: rr*ndm = 1)
The exp bias cancels between numerator and denominator and the argument is
always <= 0 (pass A and pass B matmuls are bitwise equal), so no wmax pass.

Per 128-row block: pass A matmul -> PSUM -> VectorE row max (the whole scale
chain stays on the DVE so its strict-FIFO queue never waits on another
engine); pass B matmul -> PSUM -> ScalarE Exp with accum_out giving Z.
Blocks are software-pipelined depth-2.  Prologue care: activation calls are
batched by ACT table set (all Square/Identity first, then one Ln/Exp group)
to avoid ~2.7us table reloads, input DMA uses 6 large transfers (issue cost
~0.8us each), and dummy matmuls at t=0 warm the PE HAM clock gate.

Sharding: 8 cores = 4 batch samples x 2 row-halves of 2048 rows each.
Host: loss_b = -log((core0.acc.sum + core1.acc.sum)/4096), acc = sum_nb 1/Z.
"""

import numpy as np

B, C, HW = 4, 256, 4096
HALF = HW // 2
NCORES = 8
NB = HALF // 128      # 16 row blocks per core
MT = HW // 1024       # 4 psum tiles of [128,1024] per block per pass
H_INV = 10.0          # 1/h with h = 0.1

_nc_cache = None


def _build():
    import concourse.bass as bass
    import concourse.bacc as bacc
    import concourse.tile as tile
    from concourse import mybir

    f32 = mybir.dt.float32
    bf16 = mybir.dt.bfloat16
    AF = mybir.ActivationFunctionType
    OP = mybir.AluOpType
    AX = mybir.AxisListType

    nc = bacc.Bacc(None)

    y_dram = nc.dram_tensor("y", [C, HW], f32, kind="ExternalInput")
    x_dram = nc.dram_tensor("xh", [C, HALF], f32, kind="ExternalInput")
    out_dram = nc.dram_tensor("out", [128, 1], f32, kind="ExternalOutput")
    xt_dram = nc.dram_tensor("xt_scratch", [1, HALF], f32)  # transpose bounce

    with tile.TileContext(nc) as tc:
        with (
            tc.tile_pool(name="big", bufs=1) as big,
            tc.tile_pool(name="singles", bufs=1) as singles,
            tc.tile_pool(name="rows", bufs=1) as rows,
            tc.tile_pool(name="stats", bufs=6) as stats,
            tc.tile_pool(name="dumps", bufs=2) as dumps,
        ):
            # ---------------- constants ----------------
            ones_col = singles.tile([128, 1], bf16)
            nc.vector.memset(ones_col, 1.0)
            cm1p001 = singles.tile([128, 1], f32)
            nc.vector.memset(cm1p001, -1.001)
            warm = singles.tile([128, 512], bf16)
            nc.vector.memset(warm, 0.0)

            # ---------------- PE warm-up (HAM clock gate) ----------------
            # ~14 dummy matmuls keep the PE busy >3.4us so the 2.4GHz clock
            # engages before the real sum-of-squares matmuls arrive.
            with tc.tile_pool(name="psw", bufs=1, space="PSUM") as psw:
                wps = psw.tile([1, 512], f32)
                for i in range(14):
                    nc.tensor.matmul(wps, ones_col, warm, start=True, stop=True)
                wdump = stats.tile([128, 1], f32, tag="wd")
                nc.vector.reduce_max(out=wdump[0:1, :], in_=wps, axis=AX.X)

            # ------------- load inputs (6 large DMAs) -------------
            y_sb = [big.tile([128, HW], f32, tag=f"y{i}", name=f"y{i}") for i in range(2)]
            x_sb = [big.tile([128, HALF], f32, tag=f"x{i}", name=f"x{i}") for i in range(2)]
            for i in range(2):
                for h in range(2):
                    sl = slice(h * 2048, (h + 1) * 2048)
                    nc.sync.dma_start(out=y_sb[i][:, sl], in_=y_dram[128 * i : 128 * (i + 1), sl])
            for i in range(2):
                nc.sync.dma_start(out=x_sb[i], in_=x_dram[128 * i : 128 * (i + 1), :])

            # ---------------- spatial mean of Y (per 2048-half) ----------------
            yn = [big.tile([128, HW], bf16, tag=f"yn{i}", name=f"yn{i}") for i in range(2)]
            ysp = [singles.tile([128, 2], f32, tag=f"ysp{i}", name=f"ysp{i}") for i in range(2)]
            scratch = big.tile([128, 2048], bf16, tag="scratch")
            for h in range(2):
                sl = slice(h * 2048, (h + 1) * 2048)
                nc.vector.reduce_sum(out=ysp[0][:, h : h + 1], in_=y_sb[0][:, sl], axis=AX.X)
                nc.scalar.activation(
                    out=scratch, in_=y_sb[1][:, sl], func=AF.Identity, bias=0.0,
                    scale=1.0, accum_out=ysp[1][:, h : h + 1],
                )
            negmean = [singles.tile([128, 1], f32, tag=f"nm{i}", name=f"nm{i}") for i in range(2)]
            for i in range(2):
                ys = stats.tile([128, 1], f32, tag="ys")
                nc.vector.reduce_sum(out=ys, in_=ysp[i], axis=AX.X)
                nc.vector.tensor_scalar_mul(out=negmean[i], in0=ys, scalar1=-1.0 / HW)

            # ---- Square/Identity phase (one ACT table set) ----
            ysq = [big.tile([128, HW], bf16, tag=f"ysq{i}", name=f"ysq{i}") for i in range(2)]
            xsq = [big.tile([128, HALF], bf16, tag=f"xsq{i}", name=f"xsq{i}") for i in range(2)]
            xcb = [big.tile([128, HALF], bf16, tag=f"xcb{i}", name=f"xcb{i}") for i in range(2)]
            for c in range(4):
                sl = slice(c * 1024, (c + 1) * 1024)
                for i in range(2):
                    nc.scalar.activation(
                        out=ysq[i][:, sl], in_=y_sb[i][:, sl], func=AF.Square,
                        bias=negmean[i], scale=1.0,
                    )
            for i in range(2):
                nc.scalar.activation(
                    out=xcb[i], in_=x_sb[i], func=AF.Identity, bias=negmean[i], scale=1.0
                )
            for i in range(2):
                nc.scalar.activation(
                    out=xsq[i], in_=x_sb[i], func=AF.Square, bias=negmean[i], scale=1.0
                )

            # ---- sum-of-squares matmuls (trail the squares chunkwise) ----
            lny_row = rows.tile([1, HW], f32)
            invny_row = rows.tile([1, HW], f32)
            lnx_row = rows.tile([1, HALF], f32)
            lnx_t = singles.tile([128, NB], f32)
            g_t = singles.tile([128, NB], f32)
            gm10 = singles.tile([128, NB], f32)
            invny_b = big.tile([128, HW], f32, tag="invny_b")

            with (
                tc.tile_pool(name="psy", bufs=1, space="PSUM") as psy,
                tc.tile_pool(name="psx", bufs=1, space="PSUM") as psx,
            ):
                ssx = psx.tile([1, HALF], f32)
                for t in range(HALF // 512):
                    sl = slice(t * 512, (t + 1) * 512)
                    for i in range(2):
                        nc.tensor.matmul(
                            ssx[0:1, sl], ones_col, xsq[i][:, sl],
                            start=(i == 0), stop=(i == 1),
                        )

                def emit_y_half(hh, ssy):
                    # ss matmuls for this 2048-col half, then ln/exp/bcast/yn
                    base = hh * 2048
                    for t in range(4):
                        sl = slice(base + t * 512, base + (t + 1) * 512)
                        psl = slice(t * 512, (t + 1) * 512)
                        for i in range(2):
                            nc.tensor.matmul(
                                ssy[0:1, psl], ones_col, ysq[i][:, sl],
                                start=(i == 0), stop=(i == 1),
                            )
                    for c in range(2):
                        sl = slice(base + c * 1024, base + (c + 1) * 1024)
                        psl = slice(c * 1024, (c + 1) * 1024)
                        nc.scalar.activation(
                            out=lny_row[0:1, sl], in_=ssy[0:1, psl], func=AF.Ln,
                            bias=0.0, scale=1.0,
                        )
                        nc.scalar.activation(
                            out=invny_row[0:1, sl], in_=lny_row[0:1, sl], func=AF.Exp,
                            bias=0.0, scale=-0.5,
                        )
                        for h in range(2):
                            s2 = slice(sl.start + h * 512, sl.start + (h + 1) * 512)
                            nc.gpsimd.partition_broadcast(invny_b[:, s2], invny_row[0:1, s2])
                        for i in range(2):
                            nc.vector.scalar_tensor_tensor(
                                out=yn[i][:, sl], in0=y_sb[i][:, sl], scalar=negmean[i],
                                in1=invny_b[:, sl], op0=OP.add, op1=OP.mult,
                            )

                ssy0 = psy.tile([1, 2048], f32, tag="ssy", name="ssy0")
                # half-0 ss matmuls can start right away; its ln waits for the
                # Ln/Exp table phase below
                base_mm_half0 = ssy0
                for t in range(4):
                    sl = slice(t * 512, (t + 1) * 512)
                    for i in range(2):
                        nc.tensor.matmul(
                            ssy0[0:1, sl], ones_col, ysq[i][:, sl],
                            start=(i == 0), stop=(i == 1),
                        )

                # ---- Ln/Exp phase (single natural_log_exp table set) ----
                # X first: the transpose bounce has DMA latency to hide.
                for t in range(HALF // 1024):
                    sl = slice(t * 1024, (t + 1) * 1024)
                    nc.scalar.activation(
                        out=lnx_row[0:1, sl], in_=ssx[0:1, sl], func=AF.Ln,
                        bias=0.0, scale=1.0,
                    )
                nc.gpsimd.dma_start(out=xt_dram[:, :], in_=lnx_row)
                nc.gpsimd.dma_start(
                    out=lnx_t, in_=xt_dram.rearrange("o (j p) -> (o p) j", p=128)
                )
                nc.scalar.activation(out=g_t, in_=lnx_t, func=AF.Exp, bias=0.0, scale=-0.5)
                nc.vector.tensor_scalar_mul(out=gm10, in0=g_t, scalar1=-H_INV)

                # Y half 0: ln -> exp -> broadcast -> yn
                for c in range(2):
                    sl = slice(c * 1024, (c + 1) * 1024)
                    nc.scalar.activation(
                        out=lny_row[0:1, sl], in_=ssy0[0:1, sl], func=AF.Ln,
                        bias=0.0, scale=1.0,
                    )
                    nc.scalar.activation(
                        out=invny_row[0:1, sl], in_=lny_row[0:1, sl], func=AF.Exp,
                        bias=0.0, scale=-0.5,
                    )
                    for h in range(2):
                        s2 = slice(sl.start + h * 512, sl.start + (h + 1) * 512)
                        nc.gpsimd.partition_broadcast(invny_b[:, s2], invny_row[0:1, s2])
                    for i in range(2):
                        nc.vector.scalar_tensor_tensor(
                            out=yn[i][:, sl], in0=y_sb[i][:, sl], scalar=negmean[i],
                            in1=invny_b[:, sl], op0=OP.add, op1=OP.mult,
                        )
                # Y half 1 (reuses the psy banks after half-0 lns complete)
                ssy1 = psy.tile([1, 2048], f32, tag="ssy", name="ssy1")
                emit_y_half(1, ssy1)

            # -------- main loop: depth-2 software-pipelined blocks --------
            zall = singles.tile([128, NB * MT], f32)
            scale_state = {}

            with (
                tc.tile_pool(name="psA", bufs=2, space="PSUM") as psA,
                tc.tile_pool(name="psB", bufs=2, space="PSUM") as psB,
            ):
                def emit_passA_and_scale(nb):
                    nsl = slice(nb * 128, (nb + 1) * 128)
                    mx4 = stats.tile([128, MT], f32, tag="mx4")
                    for j in range(MT):
                        pa = psA.tile([128, 1024], f32, tag="pa")
                        for kk in range(2):
                            for jj in range(2):
                                msl = slice(j * 1024 + jj * 512, j * 1024 + (jj + 1) * 512)
                                osl = slice(jj * 512, (jj + 1) * 512)
                                nc.tensor.matmul(
                                    pa[:, osl], xcb[kk][:, nsl], yn[kk][:, msl],
                                    start=(kk == 0), stop=(kk == 1),
                                )
                        nc.vector.reduce_max(out=mx4[:, j : j + 1], in_=pa, axis=AX.X)
                    smax = stats.tile([128, 1], f32, tag="smax")
                    nc.vector.reduce_max(out=smax, in_=mx4, axis=AX.X)
                    ndm = stats.tile([128, 1], f32, tag="ndm")
                    nc.vector.scalar_tensor_tensor(
                        out=ndm, in0=smax, scalar=g_t[:, nb : nb + 1], in1=cm1p001,
                        op0=OP.mult, op1=OP.add,
                    )
                    rr = stats.tile([128, 1], f32, tag="rr")
                    nc.vector.reciprocal(out=rr, in_=ndm)
                    a_col = stats.tile([128, 1], f32, tag="acol")
                    nc.vector.tensor_tensor(
                        out=a_col, in0=rr, in1=gm10[:, nb : nb + 1], op=OP.mult
                    )
                    eb = stats.tile([128, 1], f32, tag="eb")
                    nc.vector.tensor_scalar(
                        out=eb, in0=rr, scalar1=10.01, scalar2=H_INV,
                        op0=OP.mult, op1=OP.add,
                    )
                    scale_state[nb] = (a_col, eb)

                def emit_passB(nb):
                    nsl = slice(nb * 128, (nb + 1) * 128)
                    a_col, eb = scale_state.pop(nb)
                    for j in range(MT):
                        pb = psB.tile([128, 1024], f32, tag="pb")
                        for kk in range(2):
                            for jj in range(2):
                                msl = slice(j * 1024 + jj * 512, j * 1024 + (jj + 1) * 512)
                                osl = slice(jj * 512, (jj + 1) * 512)
                                nc.tensor.matmul(
                                    pb[:, osl], xcb[kk][:, nsl], yn[kk][:, msl],
                                    start=(kk == 0), stop=(kk == 1),
                                )
                        dump = dumps.tile([128, 1024], bf16, tag="dump")
                        nc.scalar.activation(
                            out=dump, in_=pb, func=AF.Exp,
                            bias=eb, scale=a_col,
                            accum_out=zall[:, nb * MT + j : nb * MT + j + 1],
                        )

                for nb in range(NB):
                    emit_passA_and_scale(nb)
                    if nb >= 2:
                        emit_passB(nb - 2)
                for nb in range(NB - 2, NB):
                    emit_passB(nb)

            # ---------------- epilogue: acc_p = sum_nb 1/Z ----------------
            zs = singles.tile([128, NB], f32)
            nc.vector.reduce_sum(
                out=zs, in_=zall.rearrange("p (nb mt) -> p nb mt", mt=MT), axis=AX.X
            )
            rz = singles.tile([128, NB], f32)
            nc.vector.reciprocal(out=rz, in_=zs)
            acc = singles.tile([128, 1], f32)
            nc.vector.reduce_sum(out=acc, in_=rz, axis=AX.X)
            nc.sync.dma_start(out=out_dram[:, :], in_=acc)

    nc.finalize()
    return nc


def _get_nc():
    global _nc_cache
    if _nc_cache is None:
        _nc_cache = _build()
    return _nc_cache


def run_cores(inputs, **kwargs):
    """Run the 8-core SPMD kernel; returns (loss[4], BassKernelResults)."""
    from concourse.bass_utils import run_bass_kernel_spmd

    nc = _get_nc()
    X = np.asarray(inputs["X_features"], dtype=np.float32).reshape(B, C, HW)
    Y = np.asarray(inputs["Y_features"], dtype=np.float32).reshape(B, C, HW)
    in_maps = []
    for core in range(NCORES):
        b, h = divmod(core, 2)
        in_maps.append(
            {
                "y": np.ascontiguousarray(Y[b]),
                "xh": np.ascontiguousarray(X[b, :, h * HALF : (h + 1) * HALF]),
            }
        )
    res = run_bass_kernel_spmd(nc, in_maps, core_ids=list(range(NCORES)), **kwargs)
    acc = np.stack(
        [res.results[i]["out"].reshape(-1).astype(np.float64) for i in range(NCORES)]
    )  # [8, 128]
    cx = acc.reshape(B, 2 * 128).sum(axis=1) / HW
    loss = (-np.log(cx)).astype(np.float32)
    return loss, res


def kernel(**inputs):
    return run_cores(inputs)[0]


# revision 16
# speedup vs baseline: 1.0542x; 1.0542x over previous
"""ContextualLoss forward on 8 trn2 NeuronCores.

Problem: X, Y [4, 256, 64, 64] f32 ->  loss [4] f32
  y_mean[c] = mean_hw(Y);  Xc = X - y_mean; Yc = Y - y_mean
  Xn, Yn: L2-normalized over C per spatial position; S = Xn^T @ Yn  [N, N], N=4096
  d = 1 - S; dmin = row min d; w = exp((1 - d/(dmin+1e-3))/0.1); A = w/rowsum(w)
  loss_b = -log(mean_n max_m A[n, m])

Key algebra (per row n), with s = Xc^T @ Yn (X centered but unnormalized,
g = 1/||Xc||, smax = row max s):
  max_m A[n,:] = 1 / sum_m exp(a_n*(s_nm - smax_n)),
  a_n = 10*g_n/(1.001 - smax_n*g_n) = rr*(-10g),  rr = 1/(smax*g - 1.001)
  bias  = -a*smax = 10 + 10.01*rr   (exact: rr*ndm = 1)
The exp bias cancels between numerator and denominator and the argument is
always <= 0 (pass A and pass B matmuls are bitwise equal), so no wmax pass.

Per 128-row block: pass A matmul -> PSUM -> VectorE row max (the whole scale
chain stays on the DVE so its strict-FIFO queue never waits on another
engine); pass B matmul -> PSUM -> ScalarE Exp with accum_out giving Z.
Blocks are software-pipelined depth-2.  Prologue care: activation calls are
batched by ACT table set (all Square/Identity first, then one Ln/Exp group)
to avoid ~2.7us table reloads, input DMA uses 6 large transfers (issue cost
~0.8us each), and dummy matmuls at t=0 warm the PE HAM clock gate.

Sharding: 8 cores = 4 batch samples x 2 row-halves of 2048 rows each.
Host: loss_b = -log((core0.acc.sum + core1.acc.sum)/4096), acc = sum_nb 1/Z.
"""

import numpy as np

B, C, HW = 4, 256, 4096
HALF = HW // 2
NCORES = 8
NB = HALF // 128      # 16 row blocks per core
MT = HW // 1024       # 4 psum tiles of [128,1024] per block per pass
H_INV = 10.0          # 1/h with h = 0.1

_nc_cache = None


def _build():
    import concourse.bass as bass
    import concourse.bacc as bacc
    import concourse.tile as tile
    from concourse import mybir

    f32 = mybir.dt.float32
    bf16 = mybir.dt.bfloat16
    AF = mybir.ActivationFunctionType
    OP = mybir.AluOpType
    AX = mybir.AxisListType

    nc = bacc.Bacc(None)

    y_dram = nc.dram_tensor("y", [C, HW], f32, kind="ExternalInput")
    x_dram = nc.dram_tensor("xh", [C, HALF], f32, kind="ExternalInput")
    out_dram = nc.dram_tensor("out", [128, 1], f32, kind="ExternalOutput")
    xt_dram = nc.dram_tensor("xt_scratch", [1, HALF], f32)  # transpose bounce

    with tile.TileContext(nc) as tc:
        with (
            tc.tile_pool(name="big", bufs=1) as big,
            tc.tile_pool(name="singles", bufs=1) as singles,
            tc.tile_pool(name="rows", bufs=1) as rows,
            tc.tile_pool(name="stats", bufs=6) as stats,
            tc.tile_pool(name="dumps", bufs=2) as dumps,
        ):
            # ---------------- constants ----------------
            ones_col = singles.tile([128, 1], bf16)
            nc.vector.memset(ones_col, 1.0)
            cm1p001 = singles.tile([128, 1], f32)
            nc.vector.memset(cm1p001, -1.001)
            warm = singles.tile([128, 512], bf16)
            nc.vector.memset(warm, 0.0)

            # ---------------- PE warm-up (HAM clock gate) ----------------
            # ~14 dummy matmuls keep the PE busy >3.4us so the 2.4GHz clock
            # engages before the real sum-of-squares matmuls arrive.
            with tc.tile_pool(name="psw", bufs=1, space="PSUM") as psw:
                wps = psw.tile([1, 512], f32)
                for i in range(14):
                    nc.tensor.matmul(wps, ones_col, warm, start=True, stop=True)
                wdump = stats.tile([128, 1], f32, tag="wd")
                nc.vector.reduce_max(out=wdump[0:1, :], in_=wps, axis=AX.X)

            # ------------- load inputs (6 large DMAs) -------------
            y_sb = [big.tile([128, HW], f32, tag=f"y{i}", name=f"y{i}") for i in range(2)]
            x_sb = [big.tile([128, HALF], f32, tag=f"x{i}", name=f"x{i}") for i in range(2)]
            for i in range(2):
                for h in range(2):
                    sl = slice(h * 2048, (h + 1) * 2048)
                    nc.sync.dma_start(out=y_sb[i][:, sl], in_=y_dram[128 * i : 128 * (i + 1), sl])
            for i in range(2):
                nc.sync.dma_start(out=x_sb[i], in_=x_dram[128 * i : 128 * (i + 1), :])

            # ---------------- spatial mean of Y (per 2048-half) ----------------
            yn = [big.tile([128, HW], bf16, tag=f"yn{i}", name=f"yn{i}") for i in range(2)]
            ysp = [singles.tile([128, 2], f32, tag=f"ysp{i}", name=f"ysp{i}") for i in range(2)]
            for h in range(2):
                sl = slice(h * 2048, (h + 1) * 2048)
                for i in range(2):
                    nc.vector.reduce_sum(out=ysp[i][:, h : h + 1], in_=y_sb[i][:, sl], axis=AX.X)
            negmean = [singles.tile([128, 1], f32, tag=f"nm{i}", name=f"nm{i}") for i in range(2)]
            for i in range(2):
                ys = stats.tile([128, 1], f32, tag="ys")
                nc.vector.reduce_sum(out=ys, in_=ysp[i], axis=AX.X)
                nc.vector.tensor_scalar_mul(out=negmean[i], in0=ys, scalar1=-1.0 / HW)

            # ---- Square/Identity phase (one ACT table set) ----
            ysq = [big.tile([128, HW], bf16, tag=f"ysq{i}", name=f"ysq{i}") for i in range(2)]
            xsq = [big.tile([128, HALF], bf16, tag=f"xsq{i}", name=f"xsq{i}") for i in range(2)]
            xcb = [big.tile([128, HALF], bf16, tag=f"xcb{i}", name=f"xcb{i}") for i in range(2)]
            for c in range(4):
                sl = slice(c * 1024, (c + 1) * 1024)
                for i in range(2):
                    nc.scalar.activation(
                        out=ysq[i][:, sl], in_=y_sb[i][:, sl], func=AF.Square,
                        bias=negmean[i], scale=1.0,
                    )
            for i in range(2):
                nc.vector.tensor_scalar_add(out=xcb[i], in0=x_sb[i], scalar1=negmean[i])
            for i in range(2):
                nc.vector.tensor_tensor(out=xsq[i], in0=xcb[i], in1=xcb[i], op=OP.mult)

            # ---- sum-of-squares matmuls (trail the squares chunkwise) ----
            lny_row = rows.tile([1, HW], f32)
            invny_row = rows.tile([1, HW], f32)
            lnx_row = rows.tile([1, HALF], f32)
            lnx_t = singles.tile([128, NB], f32)
            g_t = singles.tile([128, NB], f32)
            gm10 = singles.tile([128, NB], f32)
            invny_b = big.tile([128, HW], f32, tag="invny_b")

            with (
                tc.tile_pool(name="psy", bufs=1, space="PSUM") as psy,
                tc.tile_pool(name="psx", bufs=1, space="PSUM") as psx,
            ):
                ssx = psx.tile([1, HALF], f32)
                for t in range(HALF // 512):
                    sl = slice(t * 512, (t + 1) * 512)
                    for i in range(2):
                        nc.tensor.matmul(
                            ssx[0:1, sl], ones_col, xsq[i][:, sl],
                            start=(i == 0), stop=(i == 1),
                        )

                def emit_y_half(hh, ssy):
                    # ss matmuls for this 2048-col half, then ln/exp/bcast/yn
                    base = hh * 2048
                    for t in range(4):
                        sl = slice(base + t * 512, base + (t + 1) * 512)
                        psl = slice(t * 512, (t + 1) * 512)
                        for i in range(2):
                            nc.tensor.matmul(
                                ssy[0:1, psl], ones_col, ysq[i][:, sl],
                                start=(i == 0), stop=(i == 1),
                            )
                    for c in range(2):
                        sl = slice(base + c * 1024, base + (c + 1) * 1024)
                        psl = slice(c * 1024, (c + 1) * 1024)
                        nc.scalar.activation(
                            out=lny_row[0:1, sl], in_=ssy[0:1, psl], func=AF.Ln,
                            bias=0.0, scale=1.0,
                        )
                        nc.scalar.activation(
                            out=invny_row[0:1, sl], in_=lny_row[0:1, sl], func=AF.Exp,
                            bias=0.0, scale=-0.5,
                        )
                        for h in range(2):
                            s2 = slice(sl.start + h * 512, sl.start + (h + 1) * 512)
                            nc.gpsimd.partition_broadcast(invny_b[:, s2], invny_row[0:1, s2])
                        for i in range(2):
                            nc.vector.scalar_tensor_tensor(
                                out=yn[i][:, sl], in0=y_sb[i][:, sl], scalar=negmean[i],
                                in1=invny_b[:, sl], op0=OP.add, op1=OP.mult,
                            )

                ssy0 = psy.tile([1, 2048], f32, tag="ssy", name="ssy0")
                # half-0 ss matmuls can start right away; its ln waits for the
                # Ln/Exp table phase below
                base_mm_half0 = ssy0
                for t in range(4):
                    sl = slice(t * 512, (t + 1) * 512)
                    for i in range(2):
                        nc.tensor.matmul(
                            ssy0[0:1, sl], ones_col, ysq[i][:, sl],
                            start=(i == 0), stop=(i == 1),
                        )

                # ---- Ln/Exp phase (single natural_log_exp table set) ----
                # X first: the transpose bounce has DMA latency to hide.
                for t in range(HALF // 1024):
                    sl = slice(t * 1024, (t + 1) * 1024)
                    nc.scalar.activation(
                        out=lnx_row[0:1, sl], in_=ssx[0:1, sl], func=AF.Ln,
                        bias=0.0, scale=1.0,
                    )
                nc.gpsimd.dma_start(out=xt_dram[:, :], in_=lnx_row)
                nc.gpsimd.dma_start(
                    out=lnx_t, in_=xt_dram.rearrange("o (j p) -> (o p) j", p=128)
                )
                nc.scalar.activation(out=g_t, in_=lnx_t, func=AF.Exp, bias=0.0, scale=-0.5)
                nc.vector.tensor_scalar_mul(out=gm10, in0=g_t, scalar1=-H_INV)

                # Y half 0: ln -> exp -> broadcast -> yn
                for c in range(2):
                    sl = slice(c * 1024, (c + 1) * 1024)
                    nc.scalar.activation(
                        out=lny_row[0:1, sl], in_=ssy0[0:1, sl], func=AF.Ln,
                        bias=0.0, scale=1.0,
                    )
                    nc.scalar.activation(
                        out=invny_row[0:1, sl], in_=lny_row[0:1, sl], func=AF.Exp,
                        bias=0.0, scale=-0.5,
                    )
                    for h in range(2):
                        s2 = slice(sl.start + h * 512, sl.start + (h + 1) * 512)
                        nc.gpsimd.partition_broadcast(invny_b[:, s2], invny_row[0:1, s2])
                    for i in range(2):
                        nc.vector.scalar_tensor_tensor(
                            out=yn[i][:, sl], in0=y_sb[i][:, sl], scalar=negmean[i],
                            in1=invny_b[:, sl], op0=OP.add, op1=OP.mult,
                        )
                # Y half 1 (reuses the psy banks after half-0 lns complete)
                ssy1 = psy.tile([1, 2048], f32, tag="ssy", name="ssy1")
                emit_y_half(1, ssy1)

            # -------- main loop: depth-2 software-pipelined blocks --------
            zall = singles.tile([128, NB * 2], f32)
            scale_state = {}

            with (
                tc.tile_pool(name="psA", bufs=2, space="PSUM") as psA,
                tc.tile_pool(name="psB", bufs=1, space="PSUM") as psB,
            ):
                def emit_passA_and_scale(nb):
                    nsl = slice(nb * 128, (nb + 1) * 128)
                    mx4 = stats.tile([128, MT], f32, tag="mx4")
                    for j in range(MT):
                        pa = psA.tile([128, 1024], f32, tag="pa")
                        for jj in range(2):
                            msl = slice(j * 1024 + jj * 512, j * 1024 + (jj + 1) * 512)
                            osl = slice(jj * 512, (jj + 1) * 512)
                            nc.tensor.matmul(
                                pa[:, osl], xcb[0][:, nsl], yn[0][:, msl],
                                start=True, stop=False,
                            )
                            nc.tensor.matmul(
                                pa[:, osl], xcb[1][:, nsl], yn[1][:, msl],
                                start=False, stop=True,
                            )
                        nc.vector.reduce_max(out=mx4[:, j : j + 1], in_=pa, axis=AX.X)
                    smax = stats.tile([128, 1], f32, tag="smax")
                    nc.vector.reduce_max(out=smax, in_=mx4, axis=AX.X)
                    ndm = stats.tile([128, 1], f32, tag="ndm")
                    nc.vector.scalar_tensor_tensor(
                        out=ndm, in0=smax, scalar=g_t[:, nb : nb + 1], in1=cm1p001,
                        op0=OP.mult, op1=OP.add,
                    )
                    rr = stats.tile([128, 1], f32, tag="rr")
                    nc.vector.reciprocal(out=rr, in_=ndm)
                    a_col = stats.tile([128, 1], f32, tag="acol")
                    nc.vector.tensor_tensor(
                        out=a_col, in0=rr, in1=gm10[:, nb : nb + 1], op=OP.mult
                    )
                    eb = stats.tile([128, 1], f32, tag="eb")
                    nc.vector.tensor_scalar(
                        out=eb, in0=rr, scalar1=10.01, scalar2=H_INV,
                        op0=OP.mult, op1=OP.add,
                    )
                    scale_state[nb] = (a_col, eb)

                def emit_passB(nb):
                    nsl = slice(nb * 128, (nb + 1) * 128)
                    a_col, eb = scale_state.pop(nb)
                    for j in range(2):
                        pb = psB.tile([128, 2048], f32, tag="pb")
                        for jj in range(4):
                            msl = slice(j * 2048 + jj * 512, j * 2048 + (jj + 1) * 512)
                            osl = slice(jj * 512, (jj + 1) * 512)
                            nc.tensor.matmul(
                                pb[:, osl], xcb[0][:, nsl], yn[0][:, msl],
                                start=True, stop=False,
                            )
                            nc.tensor.matmul(
                                pb[:, osl], xcb[1][:, nsl], yn[1][:, msl],
                                start=False, stop=True,
                            )
                        dump = dumps.tile([128, 2048], bf16, tag="dump")
                        nc.scalar.activation(
                            out=dump, in_=pb, func=AF.Exp,
                            bias=eb, scale=a_col,
                            accum_out=zall[:, nb * 2 + j : nb * 2 + j + 1],
                        )

                for nb in range(NB):
                    emit_passA_and_scale(nb)
                    if nb >= 2:
                        emit_passB(nb - 2)
                for nb in range(NB - 2, NB):
                    emit_passB(nb)

            # ---------------- epilogue: acc_p = sum_nb 1/Z ----------------
            zs = singles.tile([128, NB], f32)
            nc.vector.reduce_sum(
                out=zs, in_=zall.rearrange("p (nb nt) -> p nb nt", nt=2), axis=AX.X
            )
            rz = singles.tile([128, NB], f32)
            nc.vector.reciprocal(out=rz, in_=zs)
            acc = singles.tile([128, 1], f32)
            nc.vector.reduce_sum(out=acc, in_=rz, axis=AX.X)
            nc.sync.dma_start(out=out_dram[:, :], in_=acc)

    nc.finalize()
    return nc


def _get_nc():
    global _nc_cache
    if _nc_cache is None:
        _nc_cache = _build()
    return _nc_cache


def run_cores(inputs, **kwargs):
    """Run the 8-core SPMD kernel; returns (loss[4], BassKernelResults)."""
    from concourse.bass_utils import run_bass_kernel_spmd

    nc = _get_nc()
    X = np.asarray(inputs["X_features"], dtype=np.float32).reshape(B, C, HW)
    Y = np.asarray(inputs["Y_features"], dtype=np.float32).reshape(B, C, HW)
    in_maps = []
    for core in range(NCORES):
        b, h = divmod(core, 2)
        in_maps.append(
            {
                "y": np.ascontiguousarray(Y[b]),
                "xh": np.ascontiguousarray(X[b, :, h * HALF : (h + 1) * HALF]),
            }
        )
    res = run_bass_kernel_spmd(nc, in_maps, core_ids=list(range(NCORES)), **kwargs)
    acc = np.stack(
        [res.results[i]["out"].reshape(-1).astype(np.float64) for i in range(NCORES)]
    )  # [8, 128]
    cx = acc.reshape(B, 2 * 128).sum(axis=1) / HW
    loss = (-np.log(cx)).astype(np.float32)
    return loss, res


def kernel(**inputs):
    return run_cores(inputs)[0]


# revision 18
# speedup vs baseline: 1.1877x; 1.1266x over previous
"""ContextualLoss forward on 8 trn2 NeuronCores.

Problem: X, Y [4, 256, 64, 64] f32 ->  loss [4] f32
  y_mean[c] = mean_hw(Y);  Xc = X - y_mean; Yc = Y - y_mean
  Xn, Yn: L2-normalized over C per spatial position; S = Xn^T @ Yn  [N, N], N=4096
  d = 1 - S; dmin = row min d; w = exp((1 - d/(dmin+1e-3))/0.1); A = w/rowsum(w)
  loss_b = -log(mean_n max_m A[n, m])

Key algebra (per row n), with s = Xc^T @ Yn (X centered but unnormalized,
g = 1/||Xc||, smax = row max s):
  max_m A[n,:] = 1 / sum_m exp(a_n*(s_nm - smax_n)),
  a_n = 10*g_n/(1.001 - smax_n*g_n) = rr*(-10g),  rr = 1/(smax*g - 1.001)
  bias  = -a*smax = 10 + 10.01*rr   (exact: rr*ndm = 1)
The exp bias cancels between numerator and denominator and the argument is
always <= 0 (pass A and pass B matmuls are bitwise equal), so no wmax pass.

Per 128-row block: pass A matmul -> PSUM -> VectorE row max (the whole scale
chain stays on the DVE so its strict-FIFO queue never waits on another
engine); pass B matmul -> PSUM -> ScalarE Exp with accum_out giving Z.
Blocks are software-pipelined depth-2.  Prologue care: activation calls are
batched by ACT table set (all Square/Identity first, then one Ln/Exp group)
to avoid ~2.7us table reloads, input DMA uses 6 large transfers (issue cost
~0.8us each), and dummy matmuls at t=0 warm the PE HAM clock gate.

Sharding: 8 cores = 4 batch samples x 2 row-halves of 2048 rows each.
Host: loss_b = -log((core0.acc.sum + core1.acc.sum)/4096), acc = sum_nb 1/Z.
"""

import numpy as np

B, C, HW = 4, 256, 4096
HALF = HW // 2
NCORES = 8
NB = HALF // 128      # 16 row blocks per core
MT = HW // 1024       # 4 psum tiles of [128,1024] per block per pass
H_INV = 10.0          # 1/h with h = 0.1

_nc_cache = None


def _build():
    import concourse.bass as bass
    import concourse.bacc as bacc
    import concourse.tile as tile
    from concourse import mybir

    f32 = mybir.dt.float32
    bf16 = mybir.dt.bfloat16
    f8 = mybir.dt.float8e4
    PM = mybir.MatmulPerfMode
    AF = mybir.ActivationFunctionType
    OP = mybir.AluOpType
    AX = mybir.AxisListType

    nc = bacc.Bacc(None)

    y_dram = nc.dram_tensor("y", [C, HW], f32, kind="ExternalInput")
    x_dram = nc.dram_tensor("xh", [C, HALF], f32, kind="ExternalInput")
    out_dram = nc.dram_tensor("out", [128, 1], f32, kind="ExternalOutput")
    xt_dram = nc.dram_tensor("xt_scratch", [1, HALF], f32)  # transpose bounce

    with tile.TileContext(nc) as tc:
        with (
            tc.tile_pool(name="big", bufs=1) as big,
            tc.tile_pool(name="singles", bufs=1) as singles,
            tc.tile_pool(name="rows", bufs=1) as rows,
            tc.tile_pool(name="stats", bufs=6) as stats,
            tc.tile_pool(name="dumps", bufs=2) as dumps,
        ):
            # ---------------- constants ----------------
            ones_col = singles.tile([128, 1], bf16)
            nc.vector.memset(ones_col, 1.0)
            cm1p001 = singles.tile([128, 1], f32)
            nc.vector.memset(cm1p001, -1.001)
            ln16_row = singles.tile([1, 1], f32)
            nc.vector.memset(ln16_row, 2.772588722239781)
            mln16_col = singles.tile([128, 1], f32)
            nc.vector.memset(mln16_col, -2.772588722239781)
            warm = singles.tile([128, 512], bf16)
            nc.vector.memset(warm, 0.0)

            # ---------------- PE warm-up (HAM clock gate) ----------------
            # ~14 dummy matmuls keep the PE busy >3.4us so the 2.4GHz clock
            # engages before the real sum-of-squares matmuls arrive.
            with tc.tile_pool(name="psw", bufs=1, space="PSUM") as psw:
                wps = psw.tile([1, 512], f32)
                for i in range(14):
                    nc.tensor.matmul(wps, ones_col, warm, start=True, stop=True)
                wdump = stats.tile([128, 1], f32, tag="wd")
                nc.vector.reduce_max(out=wdump[0:1, :], in_=wps, axis=AX.X)

            # ------------- load inputs (6 large DMAs) -------------
            y_sb = [big.tile([128, HW], f32, tag=f"y{i}", name=f"y{i}") for i in range(2)]
            x_sb = [big.tile([128, HALF], f32, tag=f"x{i}", name=f"x{i}") for i in range(2)]
            for i in range(2):
                for h in range(2):
                    sl = slice(h * 2048, (h + 1) * 2048)
                    nc.sync.dma_start(out=y_sb[i][:, sl], in_=y_dram[128 * i : 128 * (i + 1), sl])
            for i in range(2):
                nc.sync.dma_start(out=x_sb[i], in_=x_dram[128 * i : 128 * (i + 1), :])

            # ---------------- spatial mean of Y (per 2048-half) ----------------
            yn8 = big.tile([128, 2 * HW], f8, tag="yn8", name="yn8")
            ysp = [singles.tile([128, 2], f32, tag=f"ysp{i}", name=f"ysp{i}") for i in range(2)]
            for h in range(2):
                sl = slice(h * 2048, (h + 1) * 2048)
                for i in range(2):
                    nc.vector.reduce_sum(out=ysp[i][:, h : h + 1], in_=y_sb[i][:, sl], axis=AX.X)
            negmean = [singles.tile([128, 1], f32, tag=f"nm{i}", name=f"nm{i}") for i in range(2)]
            for i in range(2):
                ys = stats.tile([128, 1], f32, tag="ys")
                nc.vector.reduce_sum(out=ys, in_=ysp[i], axis=AX.X)
                nc.vector.tensor_scalar_mul(out=negmean[i], in0=ys, scalar1=-1.0 / HW)

            # ---- Square/Identity phase (one ACT table set) ----
            ysq = [big.tile([128, HW], bf16, tag=f"ysq{i}", name=f"ysq{i}") for i in range(2)]
            xsq = [big.tile([128, HALF], bf16, tag=f"xsq{i}", name=f"xsq{i}") for i in range(2)]
            xcb8 = big.tile([128, 2 * HALF], f8, tag="xcb8", name="xcb8")
            for c in range(4):
                sl = slice(c * 1024, (c + 1) * 1024)
                for i in range(2):
                    nc.scalar.activation(
                        out=ysq[i][:, sl], in_=y_sb[i][:, sl], func=AF.Square,
                        bias=negmean[i], scale=1.0,
                    )
            xcb = [xcb8[:, i * HALF : (i + 1) * HALF] for i in range(2)]
            for i in range(2):
                nc.vector.tensor_scalar_add(out=xcb[i], in0=x_sb[i], scalar1=negmean[i])
            for i in range(2):
                nc.scalar.activation(
                    out=xsq[i], in_=x_sb[i], func=AF.Square, bias=negmean[i], scale=1.0
                )

            # ---- sum-of-squares matmuls (trail the squares chunkwise) ----
            lny_row = rows.tile([1, HW], f32)
            invny_row = rows.tile([1, HW], f32)
            lnx_row = rows.tile([1, HALF], f32)
            lnx_t = singles.tile([128, NB], f32)
            g_t = singles.tile([128, NB], f32)
            gm10 = singles.tile([128, NB], f32)
            invny_b = big.tile([128, HW], f32, tag="invny_b")

            with (
                tc.tile_pool(name="psy", bufs=1, space="PSUM") as psy,
                tc.tile_pool(name="psx", bufs=1, space="PSUM") as psx,
            ):
                ssx = psx.tile([1, HALF], f32)
                for t in range(HALF // 512):
                    sl = slice(t * 512, (t + 1) * 512)
                    for i in range(2):
                        nc.tensor.matmul(
                            ssx[0:1, sl], ones_col, xsq[i][:, sl],
                            start=(i == 0), stop=(i == 1),
                        )

                def emit_y_half(hh, ssy):
                    # ss matmuls for this 2048-col half, then ln/exp/bcast/yn
                    base = hh * 2048
                    for t in range(4):
                        sl = slice(base + t * 512, base + (t + 1) * 512)
                        psl = slice(t * 512, (t + 1) * 512)
                        for i in range(2):
                            nc.tensor.matmul(
                                ssy[0:1, psl], ones_col, ysq[i][:, sl],
                                start=(i == 0), stop=(i == 1),
                            )
                    for c in range(2):
                        sl = slice(base + c * 1024, base + (c + 1) * 1024)
                        psl = slice(c * 1024, (c + 1) * 1024)
                        nc.scalar.activation(
                            out=lny_row[0:1, sl], in_=ssy[0:1, psl], func=AF.Ln,
                            bias=0.0, scale=1.0,
                        )
                        nc.scalar.activation(
                            out=invny_row[0:1, sl], in_=lny_row[0:1, sl], func=AF.Exp,
                            bias=ln16_row, scale=-0.5,
                        )
                        for h in range(2):
                            s2 = slice(sl.start + h * 512, sl.start + (h + 1) * 512)
                            nc.gpsimd.partition_broadcast(invny_b[:, s2], invny_row[0:1, s2])
                        for i in range(2):
                            nc.vector.scalar_tensor_tensor(
                                out=yn8[:, i * HW + sl.start : i * HW + sl.stop],
                                in0=y_sb[i][:, sl], scalar=negmean[i],
                                in1=invny_b[:, sl], op0=OP.add, op1=OP.mult,
                            )

                ssy0 = psy.tile([1, 2048], f32, tag="ssy", name="ssy0")
                # half-0 ss matmuls can start right away; its ln waits for the
                # Ln/Exp table phase below
                base_mm_half0 = ssy0
                for t in range(4):
                    sl = slice(t * 512, (t + 1) * 512)
                    for i in range(2):
                        nc.tensor.matmul(
                            ssy0[0:1, sl], ones_col, ysq[i][:, sl],
                            start=(i == 0), stop=(i == 1),
                        )

                # ---- Ln/Exp phase (single natural_log_exp table set) ----
                # X first: the transpose bounce has DMA latency to hide.
                for t in range(HALF // 1024):
                    sl = slice(t * 1024, (t + 1) * 1024)
                    nc.scalar.activation(
                        out=lnx_row[0:1, sl], in_=ssx[0:1, sl], func=AF.Ln,
                        bias=0.0, scale=1.0,
                    )
                nc.gpsimd.dma_start(out=xt_dram[:, :], in_=lnx_row)
                nc.gpsimd.dma_start(
                    out=lnx_t, in_=xt_dram.rearrange("o (j p) -> (o p) j", p=128)
                )
                nc.scalar.activation(out=g_t, in_=lnx_t, func=AF.Exp, bias=mln16_col, scale=-0.5)
                nc.vector.tensor_scalar_mul(out=gm10, in0=g_t, scalar1=-H_INV)

                # Y half 0: ln -> exp -> broadcast -> yn
                for c in range(2):
                    sl = slice(c * 1024, (c + 1) * 1024)
                    nc.scalar.activation(
                        out=lny_row[0:1, sl], in_=ssy0[0:1, sl], func=AF.Ln,
                        bias=0.0, scale=1.0,
                    )
                    nc.scalar.activation(
                        out=invny_row[0:1, sl], in_=lny_row[0:1, sl], func=AF.Exp,
                        bias=ln16_row, scale=-0.5,
                    )
                    for h in range(2):
                        s2 = slice(sl.start + h * 512, sl.start + (h + 1) * 512)
                        nc.gpsimd.partition_broadcast(invny_b[:, s2], invny_row[0:1, s2])
                    for i in range(2):
                        nc.vector.scalar_tensor_tensor(
                            out=yn8[:, i * HW + sl.start : i * HW + sl.stop],
                            in0=y_sb[i][:, sl], scalar=negmean[i],
                            in1=invny_b[:, sl], op0=OP.add, op1=OP.mult,
                        )
                # Y half 1 (reuses the psy banks after half-0 lns complete)
                ssy1 = psy.tile([1, 2048], f32, tag="ssy", name="ssy1")
                emit_y_half(1, ssy1)

            # -------- main loop: depth-2 software-pipelined blocks --------
            zall = singles.tile([128, NB * 2], f32)
            scale_state = {}

            xcb3 = xcb8.rearrange("p (k m) -> p k m", k=2)
            yn3 = yn8.rearrange("p (k n) -> p k n", k=2)
            with (
                tc.tile_pool(name="psA", bufs=2, space="PSUM") as psA,
                tc.tile_pool(name="psB", bufs=1, space="PSUM") as psB,
            ):
                def emit_passA_and_scale(nb):
                    nsl = slice(nb * 128, (nb + 1) * 128)
                    mx4 = stats.tile([128, MT], f32, tag="mx4")
                    for j in range(MT):
                        pa = psA.tile([128, 1024], f32, tag="pa")
                        for jj in range(2):
                            msl = slice(j * 1024 + jj * 512, j * 1024 + (jj + 1) * 512)
                            osl = slice(jj * 512, (jj + 1) * 512)
                            nc.tensor.matmul(
                                pa[:, osl], xcb3[:, :, nsl], yn3[:, :, msl],
                                start=True, stop=True, perf_mode=PM.DoubleRow,
                            )
                        nc.vector.reduce_max(out=mx4[:, j : j + 1], in_=pa, axis=AX.X)
                    smax = stats.tile([128, 1], f32, tag="smax")
                    nc.vector.reduce_max(out=smax, in_=mx4, axis=AX.X)
                    ndm = stats.tile([128, 1], f32, tag="ndm")
                    nc.vector.scalar_tensor_tensor(
                        out=ndm, in0=smax, scalar=g_t[:, nb : nb + 1], in1=cm1p001,
                        op0=OP.mult, op1=OP.add,
                    )
                    rr = stats.tile([128, 1], f32, tag="rr")
                    nc.vector.reciprocal(out=rr, in_=ndm)
                    a_col = stats.tile([128, 1], f32, tag="acol")
                    nc.vector.tensor_tensor(
                        out=a_col, in0=rr, in1=gm10[:, nb : nb + 1], op=OP.mult
                    )
                    eb = stats.tile([128, 1], f32, tag="eb")
                    nc.vector.tensor_scalar(
                        out=eb, in0=rr, scalar1=10.01, scalar2=H_INV,
                        op0=OP.mult, op1=OP.add,
                    )
                    scale_state[nb] = (a_col, eb)

                def emit_passB(nb):
                    nsl = slice(nb * 128, (nb + 1) * 128)
                    a_col, eb = scale_state.pop(nb)
                    for j in range(2):
                        pb = psB.tile([128, 2048], f32, tag="pb")
                        for jj in range(4):
                            msl = slice(j * 2048 + jj * 512, j * 2048 + (jj + 1) * 512)
                            osl = slice(jj * 512, (jj + 1) * 512)
                            nc.tensor.matmul(
                                pb[:, osl], xcb3[:, :, nsl], yn3[:, :, msl],
                                start=True, stop=True, perf_mode=PM.DoubleRow,
                            )
                        dump = dumps.tile([128, 2048], bf16, tag="dump")
                        nc.scalar.activation(
                            out=dump, in_=pb, func=AF.Exp,
                            bias=eb, scale=a_col,
                            accum_out=zall[:, nb * 2 + j : nb * 2 + j + 1],
                        )

                for nb in range(NB):
                    emit_passA_and_scale(nb)
                    if nb >= 2:
                        emit_passB(nb - 2)
                for nb in range(NB - 2, NB):
                    emit_passB(nb)

            # ---------------- epilogue: acc_p = sum_nb 1/Z ----------------
            zs = singles.tile([128, NB], f32)
            nc.vector.reduce_sum(
                out=zs, in_=zall.rearrange("p (nb nt) -> p nb nt", nt=2), axis=AX.X
            )
            rz = singles.tile([128, NB], f32)
            nc.vector.reciprocal(out=rz, in_=zs)
            acc = singles.tile([128, 1], f32)
            nc.vector.reduce_sum(out=acc, in_=rz, axis=AX.X)
            nc.sync.dma_start(out=out_dram[:, :], in_=acc)

    nc.finalize()
    return nc


def _get_nc():
    global _nc_cache
    if _nc_cache is None:
        _nc_cache = _build()
    return _nc_cache


def run_cores(inputs, **kwargs):
    """Run the 8-core SPMD kernel; returns (loss[4], BassKernelResults)."""
    from concourse.bass_utils import run_bass_kernel_spmd

    nc = _get_nc()
    X = np.asarray(inputs["X_features"], dtype=np.float32).reshape(B, C, HW)
    Y = np.asarray(inputs["Y_features"], dtype=np.float32).reshape(B, C, HW)
    in_maps = []
    for core in range(NCORES):
        b, h = divmod(core, 2)
        in_maps.append(
            {
                "y": np.ascontiguousarray(Y[b]),
                "xh": np.ascontiguousarray(X[b, :, h * HALF : (h + 1) * HALF]),
            }
        )
    res = run_bass_kernel_spmd(nc, in_maps, core_ids=list(range(NCORES)), **kwargs)
    acc = np.stack(
        [res.results[i]["out"].reshape(-1).astype(np.float64) for i in range(NCORES)]
    )  # [8, 128]
    cx = acc.reshape(B, 2 * 128).sum(axis=1) / HW
    loss = (-np.log(cx)).astype(np.float32)
    return loss, res


def kernel(**inputs):
    return run_cores(inputs)[0]
